# revision 1
# baseline (speedup 1.0000x reference)
"""Contrastive patch loss (InfoNCE over sampled voxel patches) on 8 TRN2 NeuronCores.

Math
----
Reference computes, per patch p and batch b, cs[k,l] = <t2n[:,i_pk], t1n[:,i_pl]>
over k=512 sampled voxels (i = idx[p]), e = exp(cs/bw), then the mean over
(p,b,j) of -log(0.5*e_jj*(1/colsum_j + 1/rowsum_j) + eps).

Since every sampled voxel index lives in [0, 512), cs is a gather of the
512x512 Gram matrix G_b = t2n^T @ t1n.  With E_b = exp(G_b/bw) and c_p[s] the
multiplicity of voxel s in patch p:

    rowsum_j = (E_b @ c_p)[i_j]        colsum_j = (E_b^T @ c_p)[i_j]
    pos_j    = diag(E_b)[i_j]

so the whole (P,B,K,K) tensor never exists:

    loss = -1/(P*B*K) * sum_{b,p,s} c_p[s] *
           log(0.5*diagE_b[s]*(1/CS_b[s,p] + 1/RS_b[s,p]) + eps)

E is stored with a constant exponent offset (E' = E*e^-OFF, fp8e4m3): the
offset cancels in diag/RS and diag/CS, keeping the formula unchanged while
fitting e4m3 range.

Sharding: 8 cores = 2 batches x 4 patch-quarters; per-core partial sums are
returned as a (128,4) tile and summed on the host (no collectives).

Precision: Gram operands are fp8e4m3 (DoubleRow perf mode: 256-deep
contraction per matmul at half cycles/row); norms come from squares of the
same fp8 values (self-consistent normalization); E/E^T are fp8 with the
exponent offset; accumulation, exp/log and the loss reduction stay fp32.
Measured ~5e-5 rel err vs the fp32 reference.

Implementation notes:
- inputs packed into two row-contiguous DRAM tensors (fp8 / bf16) so the
  whole input is 2 dma_starts with one ~2KB descriptor per partition row
  instead of ~1300 1KB descriptors; DMA lands while the engines boot.
- t2-norms are produced directly in column form (per-partition sums via
  small matmuls) to feed exp's per-partition scale without transposes.
- E^T via fp8 PE transposes (stride-2 PSUM writes), PSUM->SBUF copies split
  between ACT and DVE; squares split between DVE and GpSimd.
- CS is accumulated per-t into fresh PSUM tiles (hw PSUM accumulation groups
  must be back-to-back matmuls) and summed on DVE; RS groups stay contiguous.
"""

import math
import os

import ml_dtypes
import numpy as np

import concourse.bacc as bacc
import concourse.tile as tile
from concourse import hw_specs, mybir
from concourse.bass_utils import run_bass_kernel_spmd

# Pin every ACTIVATE to the one table set that holds ln+exp+square+copy, so
# the kernel pays a single ACT_TABLE_LOAD instead of ping-ponging between the
# per-function default sets.
_PIN_SET = "natural_log_exp_and_others"
_orig_get_tables = hw_specs.get_activation_tables


def _pinned_tables(arch):
    tabs = _orig_get_tables(arch)
    return {k: (v if k == _PIN_SET else set()) for k, v in tabs.items()}


bacc.get_activation_tables = _pinned_tables

B, C, S = 2, 256, 512
P, K = 128, 512
BW = 0.05
EPS = 1e-5
N_CORES = 8
PQ = P // 4  # patches per core (32)
EOFF = 1.5  # exponent offset: E' = exp(cs/bw - EOFF); cancels in pos/sum
F32 = mybir.dt.float32
BF16 = mybir.dt.bfloat16
FP8 = mybir.dt.float8e4
DR = mybir.MatmulPerfMode.DoubleRow

PACK = os.environ.get("K_NOPACK") != "1"  # packed 2-tensor input DMA
# GpSimd tensor_tensor is ~2.4x slower than its cost model claims, which also
# misleads the tile scheduler -> keep both squares on DVE.
SQ2_GP = os.environ.get("K_SQ2GP") == "1"
HIPRI = os.environ.get("K_NOHIPRI") != "1"  # norm chain at high priority

# fp8 group layout (bytes per partition row):
#   fx 0:1024 | fy 1024:2048 | ident8 2048:2176 | ones 2176:2432
G8_W = 2432
# bf16 group layout (elements per partition row):
#   sq-src? no: cnt 0:128 | ident 128:256 | identh 256:384
G16_W = 384


def _build_program():
    nc = bacc.Bacc("TRN2", target_bir_lowering=False, debug=False, num_devices=N_CORES)

    if PACK:
        grp8 = nc.dram_tensor("grp8", [128, G8_W], FP8, kind="ExternalInput")
        grp16 = nc.dram_tensor("grp16", [128, G16_W], BF16, kind="ExternalInput")
    else:
        fx8 = nc.dram_tensor("fx8", [128, 2, S], FP8, kind="ExternalInput")
        fy8 = nc.dram_tensor("fy8", [128, 2, S], FP8, kind="ExternalInput")
        cntp = nc.dram_tensor("cntp", [128, 128], BF16, kind="ExternalInput")
        identd = nc.dram_tensor("identd", [128, 128], BF16, kind="ExternalInput")
        identd8 = nc.dram_tensor("identd8", [128, 128], FP8, kind="ExternalInput")
        identdh = nc.dram_tensor("identdh", [128, 128], BF16, kind="ExternalInput")
    partial = nc.dram_tensor("partial", [128, 4], F32, kind="ExternalOutput")

    with tile.TileContext(nc) as tc:
        with (
            tc.tile_pool(name="const", bufs=1) as const,
            tc.tile_pool(name="feat", bufs=1) as featp,
            tc.tile_pool(name="big", bufs=1) as big,
            tc.tile_pool(name="tmp", bufs=2) as tmp,
            tc.tile_pool(name="small", bufs=2) as small,
            tc.tile_pool(name="ps_g", bufs=2, space="PSUM") as ps_g,
            tc.tile_pool(name="ps_t", bufs=2, space="PSUM") as ps_t,
            tc.tile_pool(name="ps_cs", bufs=1, space="PSUM") as ps_cs,
            tc.tile_pool(name="ps_misc", bufs=1, space="PSUM") as ps_misc,
        ):
            # ---- input DMAs first: land while the engines boot ----
            if PACK:
                t8 = featp.tile([128, G8_W], FP8, name="t8", tag="t8")
                t16 = const.tile([128, G16_W], BF16, name="t16", tag="t16")
                # fy half first: it gates the longer norm chain (sq1 -> inv1)
                nc.sync.dma_start(out=t8[:, 1024:2432], in_=grp8[:, 1024:2432])
                nc.sync.dma_start(out=t8[:, 0:1024], in_=grp8[:, 0:1024])
                nc.sync.dma_start(out=t16, in_=grp16[:, :])
                fx = t8[:, 0:1024].rearrange("p (i s) -> p i s", i=2)
                fy = t8[:, 1024:2048].rearrange("p (i s) -> p i s", i=2)
                ident8 = t8[:, 2048:2176]
                ones8w = t8[:, 2176:2432].rearrange("p (i s) -> p i s", i=2)
                cnt_all = t16[:, 0:128]
                ident = t16[:, 128:256]
                identh = t16[:, 256:384]
            else:
                fx = featp.tile([128, 2, S], FP8, name="fx", tag="fx")
                fy = featp.tile([128, 2, S], FP8, name="fy", tag="fy")
                cnt_all = const.tile([128, 128], BF16, name="cnt_all", tag="cnt_all")
                ident = const.tile([128, 128], BF16, name="ident", tag="ident")
                ident8 = const.tile([128, 128], FP8, name="ident8", tag="ident8")
                identh = const.tile([128, 128], BF16, name="identh", tag="identh")
                nc.sync.dma_start(out=fx, in_=fx8[:, :, :])
                nc.sync.dma_start(out=fy, in_=fy8[:, :, :])
                nc.sync.dma_start(out=cnt_all, in_=cntp[:, :])
                nc.sync.dma_start(out=ident, in_=identd[:, :])
                nc.sync.dma_start(out=ident8, in_=identd8[:, :])
                nc.sync.dma_start(out=identh, in_=identdh[:, :])
            cnt = [cnt_all[:, PQ * t : PQ * (t + 1)] for t in range(4)]

            ones_col = const.tile([128, 1], BF16, name="ones_col", tag="ocb")
            nc.vector.memset(ones_col, 1.0)
            ones_row = const.tile([1, 128], BF16, name="ones_row", tag="ones_row")
            nc.vector.memset(ones_row, 1.0)
            eps_col = const.tile([128, 1], F32, name="eps_col", tag="eps_col")
            nc.vector.memset(eps_col, EPS)
            ln_ibw_col = const.tile([128, 1], F32, name="ln_ibw_col", tag="lbc")
            nc.vector.memset(ln_ibw_col, math.log(1.0 / BW))
            off_col = const.tile([128, 1], F32, name="off_col", tag="off_col")
            nc.vector.memset(off_col, -EOFF)

            # ---- squares from fp8 features (bf16 out); sq1 gates the longer
            # inv1 chain -> DVE; sq2 in parallel on GpSimd ----
            import contextlib

            hp = tc.high_priority if HIPRI else contextlib.nullcontext

            # fp8 squares: ss1 can then use one DoubleRow matmul (256-deep)
            # and the ss2 column sums get cheap fp8 weight loads
            sq1 = tmp.tile([128, 2, S], FP8, name="sq1", tag="sq1")
            with hp():
                nc.vector.tensor_tensor(
                    out=sq1, in0=fy, in1=fy, op=mybir.AluOpType.mult
                )
            sq2 = tmp.tile([128, 2, S], FP8, name="sq2", tag="sq2")
            if SQ2_GP:
                nc.gpsimd.tensor_tensor(
                    out=sq2, in0=fx, in1=fx, op=mybir.AluOpType.mult
                )
            else:
                nc.vector.tensor_tensor(
                    out=sq2, in0=fx, in1=fx, op=mybir.AluOpType.mult
                )

            # ---- PE: ss1 row, ss2 cols ----
            # one DoubleRow matmul: stationary = 128 fp8 ones columns (the
            # ISA rejects narrow DR operands), rows 0..63 of the output all
            # hold the same row-sum; only row 0 is consumed.
            ss1w_ps = ps_misc.tile([128, S], F32, name="ss1w_ps", tag="ss1_ps")
            with hp():
                nc.tensor.matmul(
                    out=ss1w_ps, lhsT=ones8w, rhs=sq1,
                    perf_mode=DR, start=True, stop=True,
                )
            # one PSUM bank: rs (cols 0:128), ss2 (128:132)
            mega_ps = ps_misc.tile([128, 132], F32, name="mega_ps", tag="mega_ps")
            ss2c_ps = mega_ps[:, 128:132]
            rs_ps = mega_ps[:, 0:128]
            for m in range(4):
                msl = slice(128 * m, 128 * (m + 1))
                for i in range(2):
                    nc.tensor.matmul(
                        out=ss2c_ps[:, m : m + 1], lhsT=sq2[:, i, msl],
                        rhs=ones_col, start=(i == 0), stop=(i == 1),
                    )  # fp8 lhsT x bf16 ones (proven mixed pattern)

            # Gram: one DoubleRow matmul per 128-row block (contraction 256).
            # Only the first two are emitted here; the last two go after the
            # bc matmul so the scheduler doesn't slot them ahead of it (bc
            # gates exp0 via the gsc multiply).
            def emit_gram(m):
                gp = ps_g.tile([128, S], F32, name=f"g_ps{m}", tag="g_ps")
                nc.tensor.matmul(
                    out=gp, lhsT=fx[:, :, 128 * m : 128 * (m + 1)], rhs=fy,
                    perf_mode=DR, start=True, stop=True,
                )
                return gp

            g_ps = [emit_gram(0), emit_gram(1)]

            # ---- ACT: norms (on the 128-replicated row: inv1 comes out
            # already broadcast, so no bc matmul / PSUM copies are needed) ----
            lns1 = big.tile([128, S], F32, name="lns1", tag="lns1")
            inv1b = big.tile([128, S], BF16, name="inv1b", tag="inv1b")
            with hp():
                nc.scalar.activation(
                    out=lns1, in_=ss1w_ps, func=mybir.ActivationFunctionType.Ln
                )
                nc.scalar.activation(
                    out=inv1b, in_=lns1,
                    func=mybir.ActivationFunctionType.Exp, scale=-0.5,
                )
            lnc2 = small.tile([128, 4], F32, name="lnc2", tag="lnc2")
            nc.scalar.activation(
                out=lnc2, in_=ss2c_ps, func=mybir.ActivationFunctionType.Ln
            )
            inv2bw = small.tile([128, 4], F32, name="inv2bw", tag="inv2bw")
            nc.scalar.activation(
                out=inv2bw, in_=lnc2,
                func=mybir.ActivationFunctionType.Exp,
                scale=-0.5, bias=ln_ibw_col,
            )

            g_ps.append(emit_gram(2))
            g_ps.append(emit_gram(3))

            # ---- per-tile: col-scale (DVE), exp (ACT, fp8 out) ----
            e = [
                big.tile([128, S], FP8, name=f"e_{m}", tag=f"e_{m}")
                for m in range(4)
            ]
            for m in range(4):
                g = tmp.tile([128, S], F32, name=f"gsc{m}", tag="gsc")
                nc.vector.tensor_tensor(
                    out=g, in0=g_ps[m], in1=inv1b, op=mybir.AluOpType.mult
                )
                nc.scalar.activation(
                    out=e[m], in_=g,
                    func=mybir.ActivationFunctionType.Exp,
                    scale=inv2bw[:, m : m + 1], bias=off_col,
                )

            # ---- per-m: transposes -> etm (fp8), CS t-pass, RS ----
            etm = [
                big.tile([128, S], FP8, name=f"etm_{m}", tag=f"etm_{m}")
                for m in range(4)
            ]
            dcol = small.tile([128, 4], F32, name="dcol", tag="dcol")
            cs_acc = small.tile([128, 128], F32, name="cs_acc", tag="cs_acc")

            def emit_transposes(m):
                # fp8 transpose writes PSUM with element step 2: stage in a
                # [128, S, 2] tile and use the stride-2 view as the output.
                et_full = ps_t.tile([128, S, 2], FP8, name=f"et_ps{m}", tag="et_ps")
                et_ps = et_full[:, :, 0]
                for a in range(4):
                    nc.tensor.transpose(
                        out=et_ps[:, 128 * a : 128 * (a + 1)],
                        in_=e[m][:, 128 * a : 128 * (a + 1)],
                        identity=ident8,
                    )
                # PSUM->SBUF move; GpSimd has no PSUM access, so alternate
                # the two engines that do.
                if m % 2 == 0:
                    nc.scalar.activation(
                        out=etm[m], in_=et_ps,
                        func=mybir.ActivationFunctionType.Copy,
                    )
                else:
                    nc.vector.tensor_copy(out=etm[m], in_=et_ps)

            # hw PSUM accumulation groups must be back-to-back matmuls, and
            # the CS t-passes interleave with transposes/RS: accumulate each
            # t-pass into a fresh PSUM tile and sum on DVE.
            def emit_cs_pass(t):
                ctp = ps_cs.tile([128, 128], F32, name=f"cs_t{t}", tag="cs_t")
                for mp in range(4):
                    nc.tensor.matmul(
                        out=ctp[:, PQ * mp : PQ * (mp + 1)],
                        lhsT=e[t][:, 128 * mp : 128 * (mp + 1)],
                        rhs=cnt[t], start=True, stop=True,
                    )
                if t == 0:
                    nc.vector.tensor_copy(out=cs_acc, in_=ctp)
                else:
                    nc.vector.tensor_tensor(
                        out=cs_acc, in0=cs_acc, in1=ctp, op=mybir.AluOpType.add
                    )

            def emit_rs(m):
                for a in range(4):
                    nc.tensor.matmul(
                        out=rs_ps[:, PQ * m : PQ * (m + 1)],
                        lhsT=etm[m][:, 128 * a : 128 * (a + 1)],
                        rhs=cnt[a], start=(a == 0), stop=(a == 3),
                    )

            def emit_dcol(m):
                # dcol[:, m] = 0.5 * diag(E')[msl] via masked row-sum.
                # All-SBUF, so it can run on the otherwise-idle GpSimd,
                # keeping DVE free for the gsc/copy/cs stream.
                scr = tmp.tile([128, 128], BF16, name=f"scr{m}", tag="scr")
                nc.vector.tensor_tensor(
                    out=scr, in0=e[m][:, 128 * m : 128 * (m + 1)], in1=identh,
                    op=mybir.AluOpType.mult,
                )
                nc.vector.tensor_reduce(
                    out=dcol[:, m : m + 1], in_=scr,
                    axis=mybir.AxisListType.X, op=mybir.AluOpType.add,
                )

            # per-m groups; RS(m) is delayed one group so the etm copy is done
            emit_transposes(0)
            emit_cs_pass(0)
            emit_dcol(0)
            emit_transposes(1)
            emit_cs_pass(1)
            emit_dcol(1)
            emit_rs(0)
            emit_transposes(2)
            emit_cs_pass(2)
            emit_dcol(2)
            emit_rs(1)
            emit_transposes(3)
            emit_cs_pass(3)
            emit_dcol(3)
            emit_rs(2)
            emit_rs(3)

            # ---- tail: sum_c c * ln(0.5*d*(1/RS+1/CS) + eps) ----
            # full-tile ops where possible (fewer instructions = shorter
            # dependency/semaphore tail); LN stays per-m for the d-scale.
            rinv = small.tile([128, 128], F32, name="rinv", tag="rinv")
            cinv = small.tile([128, 128], F32, name="cinv", tag="cinv")
            # RS/CS are sums of positive e-values (no 0/inf/denorm);
            # ~18-bit reciprocal is far inside the error budget
            nc.vector.reciprocal_approx_fast(out=rinv, in_=rs_ps)
            nc.vector.reciprocal_approx_fast(out=cinv, in_=cs_acc)
            ssum = small.tile([128, 128], F32, name="ssum", tag="ssum")
            nc.vector.tensor_tensor(
                out=ssum, in0=rinv, in1=cinv, op=mybir.AluOpType.add
            )
            gl = small.tile([128, 128], F32, name="gl", tag="gl")
            for m in range(4):
                mcol = slice(PQ * m, PQ * (m + 1))
                nc.scalar.activation(
                    out=gl[:, mcol], in_=ssum[:, mcol],
                    func=mybir.ActivationFunctionType.Ln,
                    scale=dcol[:, m : m + 1], bias=eps_col,
                )
            wgl = small.tile([128, 128], F32, name="wgl", tag="wgl")
            nc.vector.tensor_tensor(
                out=wgl, in0=gl, in1=cnt_all, op=mybir.AluOpType.mult
            )
            # acc kept at 4 columns: a [128,1] f32 output makes 4-byte DMA
            # descriptors, which lands the NEFF in a much slower teardown
            # path (~+5us) — 16-byte rows avoid it.
            acc = small.tile([128, 4], F32, name="acc", tag="acc")
            for m in range(4):
                nc.vector.tensor_reduce(
                    out=acc[:, m : m + 1],
                    in_=wgl[:, PQ * m : PQ * (m + 1)],
                    axis=mybir.AxisListType.X, op=mybir.AluOpType.add,
                )
            nc.sync.dma_start(out=partial[:, :], in_=acc)

    nc.compile()
    return nc


_NC = None


def _pack_inputs(t2, t1, idx):
    counts = np.zeros((P, S), np.float32)
    np.add.at(counts, (np.arange(P)[:, None], idx), 1.0)
    identf = np.eye(128, dtype=np.float32)

    in_maps = []
    for core in range(N_CORES):
        b, q = divmod(core, 4)
        f2i = np.ascontiguousarray(
            t2[b].reshape(2, 128, S).transpose(1, 0, 2).reshape(128, 1024)
        )
        f1i = np.ascontiguousarray(
            t1[b].reshape(2, 128, S).transpose(1, 0, 2).reshape(128, 1024)
        )
        cq = np.ascontiguousarray(
            counts[PQ * q : PQ * (q + 1)]
            .T.reshape(4, 128, PQ)
            .transpose(1, 0, 2)
            .reshape(128, 128)
        )
        if PACK:
            grp8 = np.concatenate(
                [f2i, f1i, identf, np.ones((128, 256), np.float32)], axis=1
            ).astype(ml_dtypes.float8_e4m3fn)
            grp16 = np.concatenate([cq, identf, 0.5 * identf], axis=1).astype(
                ml_dtypes.bfloat16
            )
            in_maps.append({"grp8": grp8, "grp16": grp16})
        else:
            in_maps.append(
                {
                    "fx8": f2i.reshape(128, 2, S).astype(ml_dtypes.float8_e4m3fn),
                    "fy8": f1i.reshape(128, 2, S).astype(ml_dtypes.float8_e4m3fn),
                    "cntp": cq.astype(ml_dtypes.bfloat16),
                    "identd": identf.astype(ml_dtypes.bfloat16),
                    "identd8": identf.astype(ml_dtypes.float8_e4m3fn),
                    "identdh": (0.5 * identf).astype(ml_dtypes.bfloat16),
                }
            )
    return in_maps


def _run(t2_feat, t1_feat, idx, trace=False, trace_kwargs=None):
    global _NC
    if _NC is None:
        _NC = _build_program()

    t2 = np.ascontiguousarray(np.asarray(t2_feat, np.float32).reshape(B, C, S))
    t1 = np.ascontiguousarray(np.asarray(t1_feat, np.float32).reshape(B, C, S))
    idx = np.asarray(idx)
    in_maps = _pack_inputs(t2, t1, idx)

    kwargs = {}
    if trace:
        kwargs = dict(trace=True, trace_kwargs=trace_kwargs or {})
    res = run_bass_kernel_spmd(_NC, in_maps, core_ids=list(range(N_CORES)), **kwargs)
    total = sum(r["partial"].sum(dtype=np.float64) for r in res.results)
    loss = -total / (P * B * K)
    return np.array(loss, dtype=np.float32), res


def kernel(t2_feat, t1_feat, idx):
    out, _ = _run(t2_feat, t1_feat, idx)
    return out



# revision 6
# speedup vs baseline: 1.3126x; 1.3126x over previous
"""Contrastive patch loss (InfoNCE over sampled voxel patches) on 8 TRN2 NeuronCores.

Math
----
Reference computes, per patch p and batch b, cs[k,l] = <t2n[:,i_pk], t1n[:,i_pl]>
over k=512 sampled voxels (i = idx[p]), e = exp(cs/bw), then the mean over
(p,b,j) of -log(0.5*e_jj*(1/colsum_j + 1/rowsum_j) + eps).

Since every sampled voxel index lives in [0, 512), cs is a gather of the
512x512 Gram matrix G_b = t2n^T @ t1n.  With E_b = exp(G_b/bw) and c_p[s] the
multiplicity of voxel s in patch p:

    colsum_j = (E_b^T @ c_p)[i_j]      rowsum_j = (E_b @ c_p)[i_j]
    pos_j    = diag(E_b)[i_j]

    loss = -1/(P*B*K) * sum_{b,p,s} c_p[s] *
           log(0.5*diagE_b[s]*(1/CS_b[s,p] + 1/RS_b[s,p]) + eps)

Sharding: 8 cores = 2 batches x 4 column-blocks of E.  Core (b,q) permutes
the voxel order so its block q comes first, then computes ONLY the 128-column
blocks it needs:

    E  col-block:  E[a-blk, 0-blk]  = exp(s2[a] (x) s1[0] * fx_a^T fyn)   (CS)
    E^T col-block: E^T[a-blk,0-blk] = exp(s1[a] (x) s2[0] * fy_a^T fxn)   (RS)

where fyn/fxn are the 128 moving columns pre-normalized (column scale folded
into the fp8 operand) and the per-partition scales s*[a] are applied by the
exp ACT.  CS[0-blk] / RS[0-blk] are then plain 4-term PSUM-accumulated
matmuls against the count columns -- no PE transposes, no gathers, and no
core ever materializes the full E.  Per-core partial sums return as a
(128,4) tile and are summed on the host (no collectives).

Precision: features fp8e4m3 (DoubleRow matmuls, 256-deep at half cycles/row);
norms from squares of the same fp8 values; E/E^T fp8 with a constant exponent
offset (cancels in pos/CS, pos/RS); accumulation, exp/log, loss fp32.
"""

import math
import os

import ml_dtypes
import numpy as np

import concourse.bacc as bacc
import concourse.tile as tile
from concourse import hw_specs, mybir
from concourse.bass_utils import run_bass_kernel_spmd

# Pin every ACTIVATE to the one table set that holds ln+exp, so the kernel
# pays a single ACT_TABLE_LOAD instead of ping-ponging between sets.
_PIN_SET = "natural_log_exp_and_others"
_orig_get_tables = hw_specs.get_activation_tables


def _pinned_tables(arch):
    tabs = _orig_get_tables(arch)
    return {k: (v if k == _PIN_SET else set()) for k, v in tabs.items()}


bacc.get_activation_tables = _pinned_tables

B, C, S = 2, 256, 512
P, K = 128, 512
BW = 0.05
EPS = 1e-5
N_CORES = 8
EOFF = 1.5  # exponent offset: E' = exp(cs/bw - EOFF); cancels in pos/sums
SF = 4.0  # fp8 scale on normalized features; ln(ibw/SF) folded into col bias
F32 = mybir.dt.float32
BF16 = mybir.dt.bfloat16
FP8 = mybir.dt.float8e4
DR = mybir.MatmulPerfMode.DoubleRow

FP8CNT = os.environ.get("K_FP8CNT") != "0"  # counts/ident as fp8 inside grp8
# tensor_tensor_reduce crashes the NEFF on hw (INTERNAL at execute) - keep off
TTR = os.environ.get("K_TTR") == "1"
GSQ = os.environ.get("K_GSQ") != "0"  # fy rest-squares on GpSimd

# grp8 layout (bytes per partition row):
#   ones 0:256 | fyq 256:512 | fxq 512:768 | fyr 768:1536 | fxr 1536:2304
#   [| cnt 2304:2816 | idh 2816:2944  when FP8CNT]
G8W = 2944 if FP8CNT else 2304
# grp16 (bf16): cnt 0:512 | idh 512:640
G16W = 640


def _build_program():
    nc = bacc.Bacc("TRN2", target_bir_lowering=False, debug=False, num_devices=N_CORES)

    grp8 = nc.dram_tensor("grp8", [128, G8W], FP8, kind="ExternalInput")
    if not FP8CNT:
        grp16 = nc.dram_tensor("grp16", [128, G16W], BF16, kind="ExternalInput")
    partial = nc.dram_tensor("partial", [128, 4], F32, kind="ExternalOutput")

    with tile.TileContext(nc) as tc:
        with (
            tc.tile_pool(name="feat", bufs=1) as featp,
            tc.tile_pool(name="big", bufs=1) as big,
            tc.tile_pool(name="small", bufs=1) as small,
            tc.tile_pool(name="ps_row", bufs=1, space="PSUM") as ps_row,
            tc.tile_pool(name="ps_e", bufs=1, space="PSUM") as ps_e,
            tc.tile_pool(name="ps_et", bufs=1, space="PSUM") as ps_et,
            tc.tile_pool(name="ps_cr", bufs=1, space="PSUM") as ps_cr,
        ):
            hp = tc.high_priority

            # ---- input DMAs first: land while the engines boot ----
            t8 = featp.tile([128, G8W], FP8, name="t8", tag="t8")
            nc.sync.dma_start(out=t8[:, 0:768], in_=grp8[:, 0:768])
            nc.sync.dma_start(out=t8[:, 768:1536], in_=grp8[:, 768:1536])
            nc.sync.dma_start(out=t8[:, 1536:G8W], in_=grp8[:, 1536:G8W])
            if not FP8CNT:
                t16 = featp.tile([128, G16W], BF16, name="t16", tag="t16")
                nc.sync.dma_start(out=t16, in_=grp16[:, :])

            # bias columns (ACT bias must be an AP)
            def bias_col(val, nm):
                t = small.tile([128, 1], F32, name=nm, tag=nm)
                nc.vector.memset(t, val)
                return t

            b_lnsf = bias_col(math.log(SF), "b_lnsf")
            b_lncol = bias_col(math.log(1.0 / BW / SF), "b_lncol")
            b_eoff = bias_col(-EOFF, "b_eoff")
            b_eps = bias_col(EPS, "b_eps")

            ones8w = t8[:, 0:256].rearrange("p (i s) -> p i s", i=2)
            fyq = t8[:, 256:512].rearrange("p (i s) -> p i s", i=2)
            fxq = t8[:, 512:768].rearrange("p (i s) -> p i s", i=2)
            fyr = t8[:, 768:1536].rearrange("p (i s) -> p i s", i=2)
            fxr = t8[:, 1536:2304].rearrange("p (i s) -> p i s", i=2)
            if FP8CNT:
                ones_col = t8[:, 0:1]
                cnt = [t8[:, 2304 + 128 * a : 2304 + 128 * (a + 1)] for a in range(4)]
                idh = t8[:, 2816:2944]
            else:
                ones_col = small.tile([128, 1], BF16, name="ones_col", tag="ocb")
                nc.vector.memset(ones_col, 1.0)
                cnt = [t16[:, 128 * a : 128 * (a + 1)] for a in range(4)]
                idh = t16[:, 512:640]

            def fxblk(a):
                return fxq if a == 0 else fxr[:, :, 128 * (a - 1) : 128 * a]

            def fyblk(a):
                return fyq if a == 0 else fyr[:, :, 128 * (a - 1) : 128 * a]

            # ---- squares of the moving-block features (fp8, feeds DR) ----
            sqbuf = big.tile([128, 2, 256], FP8, name="sqbuf", tag="sqbuf")
            with hp():
                nc.vector.tensor_tensor(
                    out=sqbuf[:, :, 0:128], in0=fyq, in1=fyq, op=mybir.AluOpType.mult
                )
                nc.vector.tensor_tensor(
                    out=sqbuf[:, :, 128:256], in0=fxq, in1=fxq, op=mybir.AluOpType.mult
                )

            # row-replicated channel sums of both q-block squares: one DR
            # matmul against an all-ones stationary
            ssrowq = ps_row.tile([128, 256], F32, name="ssrowq", tag="ssrowq")
            with hp():
                nc.tensor.matmul(
                    out=ssrowq, lhsT=ones8w, rhs=sqbuf,
                    perf_mode=DR, start=True, stop=True,
                )

            # rest-squares: sq2r (fx side) gates the E exps via the column
            # sums -> DVE; sq1r (fy side) optionally on the idle GpSimd
            sq2r = big.tile([128, 2, 384], FP8, name="sq2r", tag="sq2r")
            nc.vector.tensor_tensor(
                out=sq2r, in0=fxr, in1=fxr, op=mybir.AluOpType.mult
            )
            sq1r = big.tile([128, 2, 384], FP8, name="sq1r", tag="sq1r")
            if GSQ:
                nc.gpsimd.tensor_tensor(
                    out=sq1r, in0=fyr, in1=fyr, op=mybir.AluOpType.mult
                )
            else:
                nc.vector.tensor_tensor(
                    out=sq1r, in0=fyr, in1=fyr, op=mybir.AluOpType.mult
                )

            # ---- moving-column normalization: inv-norms broadcast along the
            # row come straight off the replicated sums (no transposes) ----
            lnrow = big.tile([128, 256], F32, name="lnrow", tag="lnrow")
            invrowq = big.tile([128, 256], BF16, name="invrowq", tag="invrowq")
            with hp():
                nc.scalar.activation(
                    out=lnrow, in_=ssrowq, func=mybir.ActivationFunctionType.Ln
                )
                nc.scalar.activation(
                    out=invrowq, in_=lnrow,
                    func=mybir.ActivationFunctionType.Exp,
                    scale=-0.5, bias=b_lnsf,
                )
            fyn = small.tile([128, 2, 128], FP8, name="fyn", tag="fyn")
            fxn = small.tile([128, 2, 128], FP8, name="fxn", tag="fxn")
            with hp():
                for i in range(2):
                    nc.vector.tensor_tensor(
                        out=fyn[:, i, :], in0=fyq[:, i, :], in1=invrowq[:, 0:128],
                        op=mybir.AluOpType.mult,
                    )
            for i in range(2):
                nc.vector.tensor_tensor(
                    out=fxn[:, i, :], in0=fxq[:, i, :], in1=invrowq[:, 128:256],
                    op=mybir.AluOpType.mult,
                )

            # ---- per-partition exp scales: column sums of the squares ----
            sscol = ps_row.tile([128, 8], F32, name="sscol", tag="sscol")

            def emit_colsums(side):  # side 0 = fx (cols 0:4), 1 = fy (4:8)
                sqq_off, sqr = (128, sq2r) if side == 0 else (0, sq1r)
                for a in range(4):
                    for i in range(2):
                        lhs = (
                            sqbuf[:, i, sqq_off : sqq_off + 128]
                            if a == 0
                            else sqr[:, i, 128 * (a - 1) : 128 * a]
                        )
                        nc.tensor.matmul(
                            out=sscol[:, 4 * side + a : 4 * side + a + 1],
                            lhsT=lhs, rhs=ones_col,
                            start=(i == 0), stop=(i == 1),
                        )

            def emit_invcol(side):
                lncol = small.tile([128, 4], F32, name=f"lncol{side}", tag=f"lnc{side}")
                invcol = small.tile([128, 4], F32, name=f"invcol{side}", tag=f"ivc{side}")
                nc.scalar.activation(
                    out=lncol, in_=sscol[:, 4 * side : 4 * side + 4],
                    func=mybir.ActivationFunctionType.Ln,
                )
                nc.scalar.activation(
                    out=invcol, in_=lncol,
                    func=mybir.ActivationFunctionType.Exp,
                    scale=-0.5, bias=b_lncol,
                )
                return invcol

            emit_colsums(0)
            inv2col = emit_invcol(0)

            # ---- Gram col-blocks (E then E^T), exp to fp8 ----
            ebank = ps_e.tile([128, 512], F32, name="ebank", tag="ebank")
            for a in range(4):
                nc.tensor.matmul(
                    out=ebank[:, 128 * a : 128 * (a + 1)], lhsT=fxblk(a), rhs=fyn,
                    perf_mode=DR, start=True, stop=True,
                )
            e8 = big.tile([128, 512], FP8, name="e8", tag="e8")
            for a in range(4):
                nc.scalar.activation(
                    out=e8[:, 128 * a : 128 * (a + 1)],
                    in_=ebank[:, 128 * a : 128 * (a + 1)],
                    func=mybir.ActivationFunctionType.Exp,
                    scale=inv2col[:, a : a + 1], bias=b_eoff,
                )

            emit_colsums(1)
            inv1col = emit_invcol(1)

            etbank = ps_et.tile([128, 512], F32, name="etbank", tag="etbank")
            for a in range(4):
                nc.tensor.matmul(
                    out=etbank[:, 128 * a : 128 * (a + 1)], lhsT=fyblk(a), rhs=fxn,
                    perf_mode=DR, start=True, stop=True,
                )
            et8 = big.tile([128, 512], FP8, name="et8", tag="et8")
            for a in range(4):
                nc.scalar.activation(
                    out=et8[:, 128 * a : 128 * (a + 1)],
                    in_=etbank[:, 128 * a : 128 * (a + 1)],
                    func=mybir.ActivationFunctionType.Exp,
                    scale=inv1col[:, a : a + 1], bias=b_eoff,
                )

            # ---- CS / RS: 4-term PSUM accumulation against count columns ----
            cs_ps = ps_cr.tile([128, 128], F32, name="cs_ps", tag="cs_ps")
            for a in range(4):
                nc.tensor.matmul(
                    out=cs_ps, lhsT=e8[:, 128 * a : 128 * (a + 1)], rhs=cnt[a],
                    start=(a == 0), stop=(a == 3),
                )
            rs_ps = ps_cr.tile([128, 128], F32, name="rs_ps", tag="rs_ps")
            for a in range(4):
                nc.tensor.matmul(
                    out=rs_ps, lhsT=et8[:, 128 * a : 128 * (a + 1)], rhs=cnt[a],
                    start=(a == 0), stop=(a == 3),
                )

            # diag: 0.5*d from the (0,0) tile via masked row-sum
            scr = small.tile([128, 128], BF16, name="scr", tag="scr")
            dcol = small.tile([128, 1], F32, name="dcol", tag="dcol")
            if TTR:
                nc.vector.tensor_tensor_reduce(
                    out=scr, in0=e8[:, 0:128], in1=idh, scale=1.0, scalar=0.0,
                    op0=mybir.AluOpType.mult, op1=mybir.AluOpType.add,
                    accum_out=dcol,
                )
            else:
                nc.vector.tensor_tensor(
                    out=scr, in0=e8[:, 0:128], in1=idh, op=mybir.AluOpType.mult
                )
                nc.vector.tensor_reduce(
                    out=dcol, in_=scr, axis=mybir.AxisListType.X,
                    op=mybir.AluOpType.add,
                )

            # ---- tail: sum_c c * ln(0.5*d*(1/RS+1/CS) + eps) ----
            # RS/CS are sums of positive e-values; ~18-bit recip is plenty
            cinv = small.tile([128, 128], F32, name="cinv", tag="cinv")
            rinv = small.tile([128, 128], F32, name="rinv", tag="rinv")
            nc.vector.reciprocal_approx_fast(out=cinv, in_=cs_ps)
            nc.vector.reciprocal_approx_fast(out=rinv, in_=rs_ps)
            ssum = small.tile([128, 128], F32, name="ssum", tag="ssum")
            nc.vector.tensor_tensor(
                out=ssum, in0=rinv, in1=cinv, op=mybir.AluOpType.add
            )
            gl = small.tile([128, 128], F32, name="gl", tag="gl")
            nc.scalar.activation(
                out=gl, in_=ssum, func=mybir.ActivationFunctionType.Ln,
                scale=dcol, bias=b_eps,
            )
            # acc kept at 4 columns: a [128,1] f32 output makes 4-byte DMA
            # descriptors, which lands the NEFF in a much slower teardown path
            acc = small.tile([128, 4], F32, name="acc", tag="acc")
            if TTR:
                wscr = small.tile([128, 128], F32, name="wscr", tag="wscr")
                for m in range(4):
                    msl = slice(32 * m, 32 * (m + 1))
                    nc.vector.tensor_tensor_reduce(
                        out=wscr[:, msl], in0=gl[:, msl], in1=cnt[0][:, msl],
                        scale=1.0, scalar=0.0,
                        op0=mybir.AluOpType.mult, op1=mybir.AluOpType.add,
                        accum_out=acc[:, m : m + 1],
                    )
            else:
                wgl = small.tile([128, 128], F32, name="wgl", tag="wgl")
                nc.vector.tensor_tensor(
                    out=wgl, in0=gl, in1=cnt[0], op=mybir.AluOpType.mult
                )
                for m in range(4):
                    nc.vector.tensor_reduce(
                        out=acc[:, m : m + 1], in_=wgl[:, 32 * m : 32 * (m + 1)],
                        axis=mybir.AxisListType.X, op=mybir.AluOpType.add,
                    )
            nc.sync.dma_start(out=partial[:, :], in_=acc)

    nc.compile()
    return nc


_NC = None


def _pack_inputs(t2, t1, idx):
    counts = np.zeros((P, S), np.float32)
    np.add.at(counts, (np.arange(P)[:, None], idx), 1.0)
    countsT = counts.T  # (S, P)
    idh = 0.5 * np.eye(128, dtype=np.float32)
    ones = np.ones((128, 256), np.float32)

    in_maps = []
    for core in range(N_CORES):
        b, q = divmod(core, 4)
        qblk = np.arange(128 * q, 128 * (q + 1))
        perm = np.concatenate([qblk, np.delete(np.arange(S), qblk)])
        # features with channel dim split (i, p): c = 128*i + p
        fx = t2[b][:, perm].reshape(2, 128, S).transpose(1, 0, 2)  # [128,2,S]
        fy = t1[b][:, perm].reshape(2, 128, S).transpose(1, 0, 2)
        cntp = countsT[perm].reshape(4, 128, 128).transpose(1, 0, 2).reshape(128, 512)
        parts = [
            ones,
            fy[:, :, 0:128].reshape(128, 256),
            fx[:, :, 0:128].reshape(128, 256),
            fy[:, :, 128:512].reshape(128, 768),
            fx[:, :, 128:512].reshape(128, 768),
        ]
        if FP8CNT:
            parts += [cntp, idh]
        grp8 = np.concatenate(parts, axis=1).astype(ml_dtypes.float8_e4m3fn)
        m = {"grp8": grp8}
        if not FP8CNT:
            m["grp16"] = np.concatenate([cntp, idh], axis=1).astype(
                ml_dtypes.bfloat16
            )
        in_maps.append(m)
    return in_maps


def _run(t2_feat, t1_feat, idx, trace=False, trace_kwargs=None):
    global _NC
    if _NC is None:
        _NC = _build_program()

    t2 = np.ascontiguousarray(np.asarray(t2_feat, np.float32).reshape(B, C, S))
    t1 = np.ascontiguousarray(np.asarray(t1_feat, np.float32).reshape(B, C, S))
    idx = np.asarray(idx)
    in_maps = _pack_inputs(t2, t1, idx)

    kwargs = {}
    if trace:
        kwargs = dict(trace=True, trace_kwargs=trace_kwargs or {})
    res = run_bass_kernel_spmd(_NC, in_maps, core_ids=list(range(N_CORES)), **kwargs)
    total = sum(r["partial"].sum(dtype=np.float64) for r in res.results)
    loss = -total / (P * B * K)
    return np.array(loss, dtype=np.float32), res


def kernel(t2_feat, t1_feat, idx):
    out, _ = _run(t2_feat, t1_feat, idx)
    return out


# revision 7
# speedup vs baseline: 1.3779x; 1.0497x over previous
"""Contrastive patch loss (InfoNCE over sampled voxel patches) on 8 TRN2 NeuronCores.

Math
----
Reference computes, per patch p and batch b, cs[k,l] = <t2n[:,i_pk], t1n[:,i_pl]>
over k=512 sampled voxels (i = idx[p]), e = exp(cs/bw), then the mean over
(p,b,j) of -log(0.5*e_jj*(1/colsum_j + 1/rowsum_j) + eps).

Since every sampled voxel index lives in [0, 512), cs is a gather of the
512x512 Gram matrix G_b = t2n^T @ t1n.  With E_b = exp(G_b/bw) and c_p[s] the
multiplicity of voxel s in patch p:

    colsum_j = (E_b^T @ c_p)[i_j]      rowsum_j = (E_b @ c_p)[i_j]
    pos_j    = diag(E_b)[i_j]

    loss = -1/(P*B*K) * sum_{b,p,s} c_p[s] *
           log(0.5*diagE_b[s]*(1/CS_b[s,p] + 1/RS_b[s,p]) + eps)

Sharding: 8 cores = 2 batches x 4 column-blocks of E.  Core (b,q) permutes
the voxel order so its block q comes first, then computes ONLY the 128-column
blocks it needs:

    E  col-block:  E[a-blk, 0-blk]  = exp(s2[a] (x) s1[0] * fx_a^T fyn)   (CS)
    E^T col-block: E^T[a-blk,0-blk] = exp(s1[a] (x) s2[0] * fy_a^T fxn)   (RS)

where fyn/fxn are the 128 moving columns pre-normalized (column scale folded
into the fp8 operand) and the per-partition scales s*[a] are applied by the
exp ACT.  CS[0-blk] / RS[0-blk] are then plain 4-term PSUM-accumulated
matmuls against the count columns -- no gathers, and no core ever
materializes the full E.  The (0,0) E^T tile is the (0,0) E tile PE-transposed
(one fewer Gram + exp).  Per-core partial sums return as a (128,4) tile and
are summed on the host (no collectives).

Engine split: DVE squares the q-block features and normalizes; ACT squares
the fx rest-blocks (idle window during input DMA) and runs all ln/exp; GpSimd
squares the fy rest-blocks; PE does channel sums, Grams, CS/RS.

Precision: features fp8e4m3 (DoubleRow matmuls, 256-deep at half cycles/row);
norms from squares of the same fp8 values; E/E^T fp8 with a constant exponent
offset (cancels in pos/CS, pos/RS); accumulation, exp/log, loss fp32.
"""

import math
import os

import ml_dtypes
import numpy as np

import concourse.bacc as bacc
import concourse.tile as tile
from concourse import hw_specs, mybir
from concourse.bass_utils import run_bass_kernel_spmd

# Pin every ACTIVATE to the one table set that holds ln+exp+square, so the
# kernel pays a single ACT_TABLE_LOAD instead of ping-ponging between sets.
_PIN_SET = "natural_log_exp_and_others"
_orig_get_tables = hw_specs.get_activation_tables


def _pinned_tables(arch):
    tabs = _orig_get_tables(arch)
    return {k: (v if k == _PIN_SET else set()) for k, v in tabs.items()}


bacc.get_activation_tables = _pinned_tables

B, C, S = 2, 256, 512
P, K = 128, 512
BW = 0.05
EPS = 1e-5
N_CORES = 8
EOFF = 1.5  # exponent offset: E' = exp(cs/bw - EOFF); cancels in pos/sums
SF = 4.0  # fp8 scale on normalized features; ln(ibw/SF) folded into col bias
F32 = mybir.dt.float32
BF16 = mybir.dt.bfloat16
FP8 = mybir.dt.float8e4
DR = mybir.MatmulPerfMode.DoubleRow

GSQ = os.environ.get("K_GSQ") != "0"  # fy rest-squares on GpSimd
ASQ = os.environ.get("K_ASQ") != "0"  # fx rest-squares on ACT
TRE = os.environ.get("K_TRE") != "0"  # E^T(0,0) tile via PE transpose

# grp8 layout (bytes per partition row):
#   ones 0:256 | fyq 256:512 | fxq 512:768 | fxr 768:1536 | fyr 1536:2304
#   | cnt 2304:2816 | idh 2816:2944 | id8 2944:3072
G8W = 3072


def _build_program():
    nc = bacc.Bacc("TRN2", target_bir_lowering=False, debug=False, num_devices=N_CORES)

    grp8 = nc.dram_tensor("grp8", [128, G8W], FP8, kind="ExternalInput")
    partial = nc.dram_tensor("partial", [128, 4], F32, kind="ExternalOutput")

    with tile.TileContext(nc) as tc:
        with (
            tc.tile_pool(name="feat", bufs=1) as featp,
            tc.tile_pool(name="big", bufs=1) as big,
            tc.tile_pool(name="small", bufs=1) as small,
            tc.tile_pool(name="ps_row", bufs=1, space="PSUM") as ps_row,
            tc.tile_pool(name="ps_e", bufs=1, space="PSUM") as ps_e,
            tc.tile_pool(name="ps_et", bufs=1, space="PSUM") as ps_et,
            tc.tile_pool(name="ps_cr", bufs=1, space="PSUM") as ps_cr,
        ):
            hp = tc.high_priority

            # ---- input DMAs first: land while the engines boot; the q-block
            # features + fx rest ride the first DMA (they gate everything) ----
            t8 = featp.tile([128, G8W], FP8, name="t8", tag="t8")
            nc.sync.dma_start(out=t8[:, 0:1536], in_=grp8[:, 0:1536])
            nc.sync.dma_start(out=t8[:, 1536:2304], in_=grp8[:, 1536:2304])
            nc.sync.dma_start(out=t8[:, 2304:G8W], in_=grp8[:, 2304:G8W])

            # bias columns (ACT bias must be an AP)
            def bias_col(val, nm):
                t = small.tile([128, 1], F32, name=nm, tag=nm)
                nc.vector.memset(t, val)
                return t

            b_lnsf = bias_col(math.log(SF), "b_lnsf")
            b_lncol = bias_col(math.log(1.0 / BW / SF), "b_lncol")
            b_eoff = bias_col(-EOFF, "b_eoff")
            b_eps = bias_col(EPS, "b_eps")

            ones8w = t8[:, 0:256].rearrange("p (i s) -> p i s", i=2)
            ones_col = t8[:, 0:1]
            fyq = t8[:, 256:512].rearrange("p (i s) -> p i s", i=2)
            fxq = t8[:, 512:768].rearrange("p (i s) -> p i s", i=2)
            fxr = t8[:, 768:1536].rearrange("p (i s) -> p i s", i=2)
            fyr = t8[:, 1536:2304].rearrange("p (i s) -> p i s", i=2)
            cnt = [t8[:, 2304 + 128 * a : 2304 + 128 * (a + 1)] for a in range(4)]
            idh = t8[:, 2816:2944]
            id8 = t8[:, 2944:3072]

            def fxblk(a):
                return fxq if a == 0 else fxr[:, :, 128 * (a - 1) : 128 * a]

            def fyblk(a):
                return fyq if a == 0 else fyr[:, :, 128 * (a - 1) : 128 * a]

            # ---- squares of the moving-block features (fp8, feeds DR) ----
            sqbuf = big.tile([128, 2, 256], FP8, name="sqbuf", tag="sqbuf")
            with hp():
                nc.vector.tensor_tensor(
                    out=sqbuf[:, :, 0:128], in0=fyq, in1=fyq, op=mybir.AluOpType.mult
                )
                nc.vector.tensor_tensor(
                    out=sqbuf[:, :, 128:256], in0=fxq, in1=fxq, op=mybir.AluOpType.mult
                )

            # fx rest-squares on ACT: it is idle while the first DMA lands,
            # and the E exps are gated on these via the fx column sums
            sq2r = big.tile([128, 2, 384], FP8, name="sq2r", tag="sq2r")
            if ASQ:
                nc.scalar.square(out=sq2r, in_=fxr)
            else:
                nc.vector.tensor_tensor(
                    out=sq2r, in0=fxr, in1=fxr, op=mybir.AluOpType.mult
                )

            # row-replicated channel sums of both q-block squares: one DR
            # matmul against an all-ones stationary
            ssrowq = ps_row.tile([128, 256], F32, name="ssrowq", tag="ssrowq")
            with hp():
                nc.tensor.matmul(
                    out=ssrowq, lhsT=ones8w, rhs=sqbuf,
                    perf_mode=DR, start=True, stop=True,
                )

            # fy rest-squares on the otherwise-idle GpSimd (gates only the
            # late E^T exps)
            sq1r = big.tile([128, 2, 384], FP8, name="sq1r", tag="sq1r")
            if GSQ:
                nc.gpsimd.tensor_tensor(
                    out=sq1r, in0=fyr, in1=fyr, op=mybir.AluOpType.mult
                )
            else:
                nc.vector.tensor_tensor(
                    out=sq1r, in0=fyr, in1=fyr, op=mybir.AluOpType.mult
                )

            # ---- moving-column normalization: inv-norms broadcast along the
            # row come straight off the replicated sums (no transposes) ----
            lnrow = big.tile([128, 256], F32, name="lnrow", tag="lnrow")
            invrowq = big.tile([128, 256], BF16, name="invrowq", tag="invrowq")
            with hp():
                nc.scalar.activation(
                    out=lnrow, in_=ssrowq, func=mybir.ActivationFunctionType.Ln
                )
                nc.scalar.activation(
                    out=invrowq, in_=lnrow,
                    func=mybir.ActivationFunctionType.Exp,
                    scale=-0.5, bias=b_lnsf,
                )
            fyn = small.tile([128, 2, 128], FP8, name="fyn", tag="fyn")
            fxn = small.tile([128, 2, 128], FP8, name="fxn", tag="fxn")
            with hp():
                for i in range(2):
                    nc.vector.tensor_tensor(
                        out=fyn[:, i, :], in0=fyq[:, i, :], in1=invrowq[:, 0:128],
                        op=mybir.AluOpType.mult,
                    )
            for i in range(2):
                nc.vector.tensor_tensor(
                    out=fxn[:, i, :], in0=fxq[:, i, :], in1=invrowq[:, 128:256],
                    op=mybir.AluOpType.mult,
                )

            # ---- E col-block Grams (gated only on fyn) ----
            ebank = ps_e.tile([128, 512], F32, name="ebank", tag="ebank")
            for a in range(4):
                nc.tensor.matmul(
                    out=ebank[:, 128 * a : 128 * (a + 1)], lhsT=fxblk(a), rhs=fyn,
                    perf_mode=DR, start=True, stop=True,
                )

            # ---- per-partition exp scales: column sums of the squares ----
            sscol = ps_row.tile([128, 8], F32, name="sscol", tag="sscol")

            def emit_colsums(side):  # side 0 = fx (cols 0:4), 1 = fy (4:8)
                sqq_off, sqr = (128, sq2r) if side == 0 else (0, sq1r)
                for a in range(4):
                    for i in range(2):
                        lhs = (
                            sqbuf[:, i, sqq_off : sqq_off + 128]
                            if a == 0
                            else sqr[:, i, 128 * (a - 1) : 128 * a]
                        )
                        nc.tensor.matmul(
                            out=sscol[:, 4 * side + a : 4 * side + a + 1],
                            lhsT=lhs, rhs=ones_col,
                            start=(i == 0), stop=(i == 1),
                        )

            def emit_invcol(side):
                lncol = small.tile([128, 4], F32, name=f"lncol{side}", tag=f"lnc{side}")
                invcol = small.tile([128, 4], F32, name=f"invcol{side}", tag=f"ivc{side}")
                nc.scalar.activation(
                    out=lncol, in_=sscol[:, 4 * side : 4 * side + 4],
                    func=mybir.ActivationFunctionType.Ln,
                )
                nc.scalar.activation(
                    out=invcol, in_=lncol,
                    func=mybir.ActivationFunctionType.Exp,
                    scale=-0.5, bias=b_lncol,
                )
                return invcol

            emit_colsums(0)
            inv2col = emit_invcol(0)

            # ---- E exps (fp8 out), CS accumulation ----
            e8 = big.tile([128, 512], FP8, name="e8", tag="e8")
            for a in range(4):
                nc.scalar.activation(
                    out=e8[:, 128 * a : 128 * (a + 1)],
                    in_=ebank[:, 128 * a : 128 * (a + 1)],
                    func=mybir.ActivationFunctionType.Exp,
                    scale=inv2col[:, a : a + 1], bias=b_eoff,
                )
            cs_ps = ps_cr.tile([128, 128], F32, name="cs_ps", tag="cs_ps")
            for a in range(4):
                nc.tensor.matmul(
                    out=cs_ps, lhsT=e8[:, 128 * a : 128 * (a + 1)], rhs=cnt[a],
                    start=(a == 0), stop=(a == 3),
                )

            # ---- E^T col-blocks: tile 0 is E(0,0) transposed; 1..3 by Gram ----
            et8 = big.tile([128, 512], FP8, name="et8", tag="et8")
            etbank = ps_et.tile([128, 512], F32, name="etbank", tag="etbank")
            a0 = 1 if TRE else 0
            if TRE:
                # fp8 transpose writes PSUM with element step 2: stage in a
                # [128, 128, 2] tile and use the stride-2 view as the output
                et_full = ps_et.tile([128, 128, 2], FP8, name="et_ps", tag="et_ps")
                nc.tensor.transpose(
                    out=et_full[:, :, 0], in_=e8[:, 0:128], identity=id8
                )
                nc.vector.tensor_copy(out=et8[:, 0:128], in_=et_full[:, :, 0])
            for a in range(a0, 4):
                nc.tensor.matmul(
                    out=etbank[:, 128 * a : 128 * (a + 1)], lhsT=fyblk(a), rhs=fxn,
                    perf_mode=DR, start=True, stop=True,
                )

            emit_colsums(1)
            inv1col = emit_invcol(1)

            for a in range(a0, 4):
                nc.scalar.activation(
                    out=et8[:, 128 * a : 128 * (a + 1)],
                    in_=etbank[:, 128 * a : 128 * (a + 1)],
                    func=mybir.ActivationFunctionType.Exp,
                    scale=inv1col[:, a : a + 1], bias=b_eoff,
                )
            rs_ps = ps_cr.tile([128, 128], F32, name="rs_ps", tag="rs_ps")
            for a in range(4):
                nc.tensor.matmul(
                    out=rs_ps, lhsT=et8[:, 128 * a : 128 * (a + 1)], rhs=cnt[a],
                    start=(a == 0), stop=(a == 3),
                )

            # diag: 0.5*d from the (0,0) tile via masked row-sum
            scr = small.tile([128, 128], BF16, name="scr", tag="scr")
            dcol = small.tile([128, 1], F32, name="dcol", tag="dcol")
            nc.vector.tensor_tensor(
                out=scr, in0=e8[:, 0:128], in1=idh, op=mybir.AluOpType.mult
            )
            nc.vector.tensor_reduce(
                out=dcol, in_=scr, axis=mybir.AxisListType.X, op=mybir.AluOpType.add
            )

            # ---- tail: sum_c c * ln(0.5*d*(1/RS+1/CS) + eps) ----
            # RS/CS are sums of positive e-values; ~18-bit recip is plenty
            cinv = small.tile([128, 128], F32, name="cinv", tag="cinv")
            rinv = small.tile([128, 128], F32, name="rinv", tag="rinv")
            nc.vector.reciprocal_approx_fast(out=cinv, in_=cs_ps)
            nc.vector.reciprocal_approx_fast(out=rinv, in_=rs_ps)
            ssum = small.tile([128, 128], F32, name="ssum", tag="ssum")
            nc.vector.tensor_tensor(
                out=ssum, in0=rinv, in1=cinv, op=mybir.AluOpType.add
            )
            gl = small.tile([128, 128], F32, name="gl", tag="gl")
            nc.scalar.activation(
                out=gl, in_=ssum, func=mybir.ActivationFunctionType.Ln,
                scale=dcol, bias=b_eps,
            )
            # acc kept at 4 columns: a [128,1] f32 output makes 4-byte DMA
            # descriptors, which lands the NEFF in a much slower teardown path
            wgl = small.tile([128, 128], F32, name="wgl", tag="wgl")
            nc.vector.tensor_tensor(
                out=wgl, in0=gl, in1=cnt[0], op=mybir.AluOpType.mult
            )
            acc = small.tile([128, 4], F32, name="acc", tag="acc")
            for m in range(4):
                nc.vector.tensor_reduce(
                    out=acc[:, m : m + 1], in_=wgl[:, 32 * m : 32 * (m + 1)],
                    axis=mybir.AxisListType.X, op=mybir.AluOpType.add,
                )
            nc.sync.dma_start(out=partial[:, :], in_=acc)

    nc.compile()
    return nc


_NC = None


def _pack_inputs(t2, t1, idx):
    counts = np.zeros((P, S), np.float32)
    np.add.at(counts, (np.arange(P)[:, None], idx), 1.0)
    countsT = counts.T  # (S, P)
    idh = 0.5 * np.eye(128, dtype=np.float32)
    id8 = np.eye(128, dtype=np.float32)
    ones = np.ones((128, 256), np.float32)

    in_maps = []
    for core in range(N_CORES):
        b, q = divmod(core, 4)
        qblk = np.arange(128 * q, 128 * (q + 1))
        perm = np.concatenate([qblk, np.delete(np.arange(S), qblk)])
        # features with channel dim split (i, p): c = 128*i + p
        fx = t2[b][:, perm].reshape(2, 128, S).transpose(1, 0, 2)  # [128,2,S]
        fy = t1[b][:, perm].reshape(2, 128, S).transpose(1, 0, 2)
        cntp = countsT[perm].reshape(4, 128, 128).transpose(1, 0, 2).reshape(128, 512)
        grp8 = np.concatenate(
            [
                ones,
                fy[:, :, 0:128].reshape(128, 256),
                fx[:, :, 0:128].reshape(128, 256),
                fx[:, :, 128:512].reshape(128, 768),
                fy[:, :, 128:512].reshape(128, 768),
                cntp,
                idh,
                id8,
            ],
            axis=1,
        ).astype(ml_dtypes.float8_e4m3fn)
        in_maps.append({"grp8": grp8})
    return in_maps


def _run(t2_feat, t1_feat, idx, trace=False, trace_kwargs=None):
    global _NC
    if _NC is None:
        _NC = _build_program()

    t2 = np.ascontiguousarray(np.asarray(t2_feat, np.float32).reshape(B, C, S))
    t1 = np.ascontiguousarray(np.asarray(t1_feat, np.float32).reshape(B, C, S))
    idx = np.asarray(idx)
    in_maps = _pack_inputs(t2, t1, idx)

    kwargs = {}
    if trace:
        kwargs = dict(trace=True, trace_kwargs=trace_kwargs or {})
    res = run_bass_kernel_spmd(_NC, in_maps, core_ids=list(range(N_CORES)), **kwargs)
    total = sum(r["partial"].sum(dtype=np.float64) for r in res.results)
    loss = -total / (P * B * K)
    return np.array(loss, dtype=np.float32), res


def kernel(t2_feat, t1_feat, idx):
    out, _ = _run(t2_feat, t1_feat, idx)
    return out


# revision 9
# speedup vs baseline: 1.4433x; 1.0475x over previous
"""Contrastive patch loss (InfoNCE over sampled voxel patches) on 8 TRN2 NeuronCores.

Math
----
Reference computes, per patch p and batch b, cs[k,l] = <t2n[:,i_pk], t1n[:,i_pl]>
over k=512 sampled voxels (i = idx[p]), e = exp(cs/bw), then the mean over
(p,b,j) of -log(0.5*e_jj*(1/colsum_j + 1/rowsum_j) + eps).

Since every sampled voxel index lives in [0, 512), cs is a gather of the
512x512 Gram matrix G_b = t2n^T @ t1n.  With E_b = exp(G_b/bw) and c_p[s] the
multiplicity of voxel s in patch p:

    colsum_j = (E_b^T @ c_p)[i_j]      rowsum_j = (E_b @ c_p)[i_j]
    pos_j    = diag(E_b)[i_j]

    loss = -1/(P*B*K) * sum_{b,p,s} c_p[s] *
           log(0.5*diagE_b[s]*(1/CS_b[s,p] + 1/RS_b[s,p]) + eps)

Sharding: 8 cores = 2 batches x 4 column-blocks of E.  Core (b,q) permutes
the voxel order so its block q comes first, then computes ONLY the 128-column
blocks it needs:

    E  col-block:  E[a-blk, 0-blk]  = exp(s2[a] (x) s1[0] * fx_a^T fyn)   (CS)
    E^T col-block: E^T[a-blk,0-blk] = exp(s1[a] (x) s2[0] * fy_a^T fxn)   (RS)

where fyn/fxn are the 128 moving columns pre-normalized (column scale folded
into the fp8 operand) and the per-partition scales s*[a] are applied by the
exp ACT.  CS[0-blk] / RS[0-blk] are then plain 4-term PSUM-accumulated
matmuls against the count columns -- no gathers, and no core ever
materializes the full E.  The (0,0) E^T tile is the (0,0) E tile PE-transposed
(one fewer Gram + exp).  Per-core partial sums return as a (128,4) tile and
are summed on the host (no collectives).

Engine split: DVE squares the q-block features (one 4-D strided TT) and
normalizes; ACT squares the fx rest-blocks (idle window during input DMA) and
runs all ln/exp; GpSimd squares the fy rest-blocks; PE does channel sums,
Grams, CS/RS.  E/E^T tiles live in per-block SBUF tiles so the exp stream is
not false-serialized against the CS/RS reads (tile-granular WAR deps).

Precision: features fp8e4m3 (DoubleRow matmuls, 256-deep at half cycles/row);
norms from squares of the same fp8 values; E/E^T fp8 with a constant exponent
offset (cancels in pos/CS, pos/RS); accumulation, exp/log, loss fp32.
"""

import math
import os

import ml_dtypes
import numpy as np

import concourse.bacc as bacc
import concourse.tile as tile
from concourse import hw_specs, mybir
from concourse.bass_utils import run_bass_kernel_spmd

# Pin every ACTIVATE to the one table set that holds ln+exp+square, so the
# kernel pays a single ACT_TABLE_LOAD instead of ping-ponging between sets.
_PIN_SET = "natural_log_exp_and_others"
_orig_get_tables = hw_specs.get_activation_tables


def _pinned_tables(arch):
    tabs = _orig_get_tables(arch)
    return {k: (v if k == _PIN_SET else set()) for k, v in tabs.items()}


bacc.get_activation_tables = _pinned_tables

B, C, S = 2, 256, 512
P, K = 128, 512
BW = 0.05
EPS = 1e-5
N_CORES = 8
EOFF = 1.5  # exponent offset: E' = exp(cs/bw - EOFF); cancels in pos/sums
SF = 4.0  # fp8 scale on normalized features; ln(ibw/SF) folded into col bias
F32 = mybir.dt.float32
BF16 = mybir.dt.bfloat16
FP8 = mybir.dt.float8e4
DR = mybir.MatmulPerfMode.DoubleRow

GSQ = os.environ.get("K_GSQ") != "0"  # fy rest-squares on GpSimd
ASQ = os.environ.get("K_ASQ") != "0"  # fx rest-squares on ACT
TRE = os.environ.get("K_TRE") != "0"  # E^T(0,0) tile via PE transpose

# grp8 layout (bytes per partition row):
#   ones 0:256 | fyq 256:512 | fxq 512:768 | fxr 768:1536 | fyr 1536:2304
#   | cnt 2304:2816 | idh 2816:2944 | id8 2944:3072
G8W = 3072


def _build_program():
    nc = bacc.Bacc("TRN2", target_bir_lowering=False, debug=False, num_devices=N_CORES)

    grp8 = nc.dram_tensor("grp8", [128, G8W], FP8, kind="ExternalInput")
    partial = nc.dram_tensor("partial", [128, 4], F32, kind="ExternalOutput")

    with tile.TileContext(nc) as tc:
        with (
            tc.tile_pool(name="feat", bufs=1) as featp,
            tc.tile_pool(name="big", bufs=1) as big,
            tc.tile_pool(name="small", bufs=1) as small,
            tc.tile_pool(name="ps_row", bufs=1, space="PSUM") as ps_row,
            tc.tile_pool(name="ps_e", bufs=1, space="PSUM") as ps_e,
            tc.tile_pool(name="ps_et", bufs=1, space="PSUM") as ps_et,
            tc.tile_pool(name="ps_cr", bufs=1, space="PSUM") as ps_cr,
        ):
            hp = tc.high_priority

            # ---- input DMAs first: land while the engines boot; the q-block
            # features + fx rest ride the first DMA (they gate everything) ----
            t8 = featp.tile([128, G8W], FP8, name="t8", tag="t8")
            nc.sync.dma_start(out=t8[:, 0:1536], in_=grp8[:, 0:1536])
            nc.sync.dma_start(out=t8[:, 1536:2304], in_=grp8[:, 1536:2304])
            nc.sync.dma_start(out=t8[:, 2304:G8W], in_=grp8[:, 2304:G8W])

            # bias columns (ACT bias must be an AP)
            def bias_col(val, nm):
                t = small.tile([128, 1], F32, name=nm, tag=nm)
                nc.vector.memset(t, val)
                return t

            b_lnsf = bias_col(math.log(SF), "b_lnsf")
            b_lncol = bias_col(math.log(1.0 / BW / SF), "b_lncol")
            b_eoff = bias_col(-EOFF, "b_eoff")
            b_eps = bias_col(EPS, "b_eps")

            ones8w = t8[:, 0:256].rearrange("p (i s) -> p i s", i=2)
            ones_col = t8[:, 0:1]
            fyq = t8[:, 256:512].rearrange("p (i s) -> p i s", i=2)
            fxq = t8[:, 512:768].rearrange("p (i s) -> p i s", i=2)
            fxr = t8[:, 768:1536].rearrange("p (i s) -> p i s", i=2)
            fyr = t8[:, 1536:2304].rearrange("p (i s) -> p i s", i=2)
            cnt = [t8[:, 2304 + 128 * a : 2304 + 128 * (a + 1)] for a in range(4)]
            idh = t8[:, 2816:2944]
            id8 = t8[:, 2944:3072]

            def fxblk(a):
                return fxq if a == 0 else fxr[:, :, 128 * (a - 1) : 128 * a]

            def fyblk(a):
                return fyq if a == 0 else fyr[:, :, 128 * (a - 1) : 128 * a]

            # ---- q-block squares: ONE 4-D strided TT over fyq+fxq.
            # sqbuf free layout (i, side, s): [fy_i0 | fx_i0 | fy_i1 | fx_i1]
            # so the DR view (i, c) has c = (side, s) matching ssrow cols ----
            sqbuf = big.tile([128, 512], FP8, name="sqbuf", tag="sqbuf")
            sq4 = sqbuf.rearrange("p (i j s) -> p i j s", i=2, j=2)
            with hp():
                nc.vector.tensor_tensor(
                    out=sq4[:, :, 0], in0=fyq, in1=fyq, op=mybir.AluOpType.mult
                )
                nc.vector.tensor_tensor(
                    out=sq4[:, :, 1], in0=fxq, in1=fxq, op=mybir.AluOpType.mult
                )

            def sq_q(side, i):  # [128,128] view of the q-block squares
                return sqbuf[:, 256 * i + 128 * side : 256 * i + 128 * (side + 1)]

            # fx rest-squares on ACT: it is idle while the first DMA lands
            sq2r = big.tile([128, 2, 384], FP8, name="sq2r", tag="sq2r")
            if ASQ:
                nc.scalar.square(out=sq2r, in_=fxr)
            else:
                nc.vector.tensor_tensor(
                    out=sq2r, in0=fxr, in1=fxr, op=mybir.AluOpType.mult
                )

            # row-replicated channel sums of both q-block squares: one DR
            # matmul against an all-ones stationary; cols 0:128 fy, 128:256 fx
            ssrowq = ps_row.tile([128, 256], F32, name="ssrowq", tag="ssrowq")
            with hp():
                nc.tensor.matmul(
                    out=ssrowq,
                    lhsT=ones8w,
                    rhs=sqbuf.rearrange("p (i c) -> p i c", i=2),
                    perf_mode=DR, start=True, stop=True,
                )

            # fy rest-squares on the otherwise-idle GpSimd (gates only the
            # late E^T exps)
            sq1r = big.tile([128, 2, 384], FP8, name="sq1r", tag="sq1r")
            if GSQ:
                nc.gpsimd.tensor_tensor(
                    out=sq1r, in0=fyr, in1=fyr, op=mybir.AluOpType.mult
                )
            else:
                nc.vector.tensor_tensor(
                    out=sq1r, in0=fyr, in1=fyr, op=mybir.AluOpType.mult
                )

            # ---- moving-column normalization, fy side first (it gates every
            # E Gram); inv-norms come out row-replicated, no transposes ----
            lnrow = big.tile([128, 256], F32, name="lnrow", tag="lnrow")
            invrowq = big.tile([128, 256], BF16, name="invrowq", tag="invrowq")
            fyn = small.tile([128, 2, 128], FP8, name="fyn", tag="fyn")
            fxn = small.tile([128, 2, 128], FP8, name="fxn", tag="fxn")
            with hp():
                nc.scalar.activation(
                    out=lnrow[:, 0:128], in_=ssrowq[:, 0:128],
                    func=mybir.ActivationFunctionType.Ln,
                )
                nc.scalar.activation(
                    out=invrowq[:, 0:128], in_=lnrow[:, 0:128],
                    func=mybir.ActivationFunctionType.Exp,
                    scale=-0.5, bias=b_lnsf,
                )
                for i in range(2):
                    nc.vector.tensor_tensor(
                        out=fyn[:, i, :], in0=fyq[:, i, :], in1=invrowq[:, 0:128],
                        op=mybir.AluOpType.mult,
                    )
            nc.scalar.activation(
                out=lnrow[:, 128:256], in_=ssrowq[:, 128:256],
                func=mybir.ActivationFunctionType.Ln,
            )
            nc.scalar.activation(
                out=invrowq[:, 128:256], in_=lnrow[:, 128:256],
                func=mybir.ActivationFunctionType.Exp,
                scale=-0.5, bias=b_lnsf,
            )
            for i in range(2):
                nc.vector.tensor_tensor(
                    out=fxn[:, i, :], in0=fxq[:, i, :], in1=invrowq[:, 128:256],
                    op=mybir.AluOpType.mult,
                )

            # ---- E col-block Grams (gated only on fyn) ----
            ebank = ps_e.tile([128, 512], F32, name="ebank", tag="ebank")
            for a in range(4):
                nc.tensor.matmul(
                    out=ebank[:, 128 * a : 128 * (a + 1)], lhsT=fxblk(a), rhs=fyn,
                    perf_mode=DR, start=True, stop=True,
                )

            # ---- per-partition exp scales: column sums of the squares ----
            sscol = ps_row.tile([128, 8], F32, name="sscol", tag="sscol")

            def emit_colsums(side):  # side 0 = fx (cols 0:4), 1 = fy (4:8)
                sqr = sq2r if side == 0 else sq1r
                for a in range(4):
                    for i in range(2):
                        lhs = (
                            sq_q(side ^ 1, i)  # fy is side 0 in sqbuf
                            if a == 0
                            else sqr[:, i, 128 * (a - 1) : 128 * a]
                        )
                        nc.tensor.matmul(
                            out=sscol[:, 4 * side + a : 4 * side + a + 1],
                            lhsT=lhs, rhs=ones_col,
                            start=(i == 0), stop=(i == 1),
                        )

            def emit_invcol(side):
                lncol = small.tile([128, 4], F32, name=f"lncol{side}", tag=f"lnc{side}")
                invcol = small.tile([128, 4], F32, name=f"invcol{side}", tag=f"ivc{side}")
                nc.scalar.activation(
                    out=lncol, in_=sscol[:, 4 * side : 4 * side + 4],
                    func=mybir.ActivationFunctionType.Ln,
                )
                nc.scalar.activation(
                    out=invcol, in_=lncol,
                    func=mybir.ActivationFunctionType.Exp,
                    scale=-0.5, bias=b_lncol,
                )
                return invcol

            emit_colsums(0)
            inv2col = emit_invcol(0)

            # ---- E exps into per-block tiles (no shared-tile WAR with the
            # CS reads -> the exp stream runs back-to-back), CS accumulation ----
            e8 = [
                big.tile([128, 128], FP8, name=f"e8_{a}", tag=f"e8_{a}")
                for a in range(4)
            ]
            for a in range(4):
                nc.scalar.activation(
                    out=e8[a], in_=ebank[:, 128 * a : 128 * (a + 1)],
                    func=mybir.ActivationFunctionType.Exp,
                    scale=inv2col[:, a : a + 1], bias=b_eoff,
                )
            cs_ps = ps_cr.tile([128, 128], F32, name="cs_ps", tag="cs_ps")
            for a in range(4):
                nc.tensor.matmul(
                    out=cs_ps, lhsT=e8[a], rhs=cnt[a],
                    start=(a == 0), stop=(a == 3),
                )

            # ---- E^T col-blocks: tile 0 is E(0,0) transposed; 1..3 by Gram ----
            et8 = [
                big.tile([128, 128], FP8, name=f"et8_{a}", tag=f"et8_{a}")
                for a in range(4)
            ]
            etbank = ps_et.tile([128, 512], F32, name="etbank", tag="etbank")
            a0 = 1 if TRE else 0
            if TRE:
                # fp8 transpose writes PSUM with element step 2: stage in a
                # [128, 128, 2] tile and use the stride-2 view as the output
                et_full = ps_et.tile([128, 128, 2], FP8, name="et_ps", tag="et_ps")
                nc.tensor.transpose(
                    out=et_full[:, :, 0], in_=e8[0], identity=id8
                )
                nc.vector.tensor_copy(out=et8[0], in_=et_full[:, :, 0])
            for a in range(a0, 4):
                nc.tensor.matmul(
                    out=etbank[:, 128 * a : 128 * (a + 1)], lhsT=fyblk(a), rhs=fxn,
                    perf_mode=DR, start=True, stop=True,
                )

            emit_colsums(1)
            inv1col = emit_invcol(1)

            for a in range(a0, 4):
                nc.scalar.activation(
                    out=et8[a], in_=etbank[:, 128 * a : 128 * (a + 1)],
                    func=mybir.ActivationFunctionType.Exp,
                    scale=inv1col[:, a : a + 1], bias=b_eoff,
                )
            rs_ps = ps_cr.tile([128, 128], F32, name="rs_ps", tag="rs_ps")
            for a in range(4):
                nc.tensor.matmul(
                    out=rs_ps, lhsT=et8[a], rhs=cnt[a],
                    start=(a == 0), stop=(a == 3),
                )

            # diag: 0.5*d from the (0,0) tile via masked row-sum
            scr = small.tile([128, 128], BF16, name="scr", tag="scr")
            dcol = small.tile([128, 1], F32, name="dcol", tag="dcol")
            nc.vector.tensor_tensor(
                out=scr, in0=e8[0], in1=idh, op=mybir.AluOpType.mult
            )
            nc.vector.tensor_reduce(
                out=dcol, in_=scr, axis=mybir.AxisListType.X, op=mybir.AluOpType.add
            )

            # ---- tail: sum_c c * ln(0.5*d*(1/RS+1/CS) + eps) ----
            # RS/CS are sums of positive e-values; ~18-bit recip is plenty
            cinv = small.tile([128, 128], F32, name="cinv", tag="cinv")
            rinv = small.tile([128, 128], F32, name="rinv", tag="rinv")
            nc.vector.reciprocal_approx_fast(out=cinv, in_=cs_ps)
            nc.vector.reciprocal_approx_fast(out=rinv, in_=rs_ps)
            ssum = small.tile([128, 128], F32, name="ssum", tag="ssum")
            nc.vector.tensor_tensor(
                out=ssum, in0=rinv, in1=cinv, op=mybir.AluOpType.add
            )
            gl = small.tile([128, 128], F32, name="gl", tag="gl")
            nc.scalar.activation(
                out=gl, in_=ssum, func=mybir.ActivationFunctionType.Ln,
                scale=dcol, bias=b_eps,
            )
            # acc kept at 4 columns: a [128,1] f32 output makes 4-byte DMA
            # descriptors, which lands the NEFF in a much slower teardown path
            acc = small.tile([128, 4], F32, name="acc", tag="acc")
            nc.vector.memset(acc[:, 1:4], 0.0)
            wgl = small.tile([128, 128], F32, name="wgl", tag="wgl")
            nc.vector.tensor_tensor(
                out=wgl, in0=gl, in1=cnt[0], op=mybir.AluOpType.mult
            )
            nc.vector.tensor_reduce(
                out=acc[:, 0:1], in_=wgl, axis=mybir.AxisListType.X,
                op=mybir.AluOpType.add,
            )
            nc.sync.dma_start(out=partial[:, :], in_=acc)

    nc.compile()
    return nc


_NC = None


def _pack_inputs(t2, t1, idx):
    counts = np.zeros((P, S), np.float32)
    np.add.at(counts, (np.arange(P)[:, None], idx), 1.0)
    countsT = counts.T  # (S, P)
    idh = 0.5 * np.eye(128, dtype=np.float32)
    id8 = np.eye(128, dtype=np.float32)
    ones = np.ones((128, 256), np.float32)

    in_maps = []
    for core in range(N_CORES):
        b, q = divmod(core, 4)
        qblk = np.arange(128 * q, 128 * (q + 1))
        perm = np.concatenate([qblk, np.delete(np.arange(S), qblk)])
        # features with channel dim split (i, p): c = 128*i + p
        fx = t2[b][:, perm].reshape(2, 128, S).transpose(1, 0, 2)  # [128,2,S]
        fy = t1[b][:, perm].reshape(2, 128, S).transpose(1, 0, 2)
        cntp = countsT[perm].reshape(4, 128, 128).transpose(1, 0, 2).reshape(128, 512)
        grp8 = np.concatenate(
            [
                ones,
                fy[:, :, 0:128].reshape(128, 256),
                fx[:, :, 0:128].reshape(128, 256),
                fx[:, :, 128:512].reshape(128, 768),
                fy[:, :, 128:512].reshape(128, 768),
                cntp,
                idh,
                id8,
            ],
            axis=1,
        ).astype(ml_dtypes.float8_e4m3fn)
        in_maps.append({"grp8": grp8})
    return in_maps


def _run(t2_feat, t1_feat, idx, trace=False, trace_kwargs=None):
    global _NC
    if _NC is None:
        _NC = _build_program()

    t2 = np.ascontiguousarray(np.asarray(t2_feat, np.float32).reshape(B, C, S))
    t1 = np.ascontiguousarray(np.asarray(t1_feat, np.float32).reshape(B, C, S))
    idx = np.asarray(idx)
    in_maps = _pack_inputs(t2, t1, idx)

    kwargs = {}
    if trace:
        kwargs = dict(trace=True, trace_kwargs=trace_kwargs or {})
    res = run_bass_kernel_spmd(_NC, in_maps, core_ids=list(range(N_CORES)), **kwargs)
    total = sum(r["partial"].sum(dtype=np.float64) for r in res.results)
    loss = -total / (P * B * K)
    return np.array(loss, dtype=np.float32), res


def kernel(t2_feat, t1_feat, idx):
    out, _ = _run(t2_feat, t1_feat, idx)
    return out
